# revision 17
# baseline (speedup 1.0000x reference)
"""Custom LSTM-cell kernel for Trainium2, data-parallel over batch on 8 NeuronCores.

Math (per token, elementwise over dff except the two GEMMs):
    gates = Hi @ Wh + Zi @ Wz + bias         # [tok, 4*dff], gate order I|F|O|Z
    A   = F~ + Mi
    M_t = max(A, I~)
    I_t = exp(I~ - M_t) = exp(min(-(A-I~), 0))
    F_t = exp(A  - M_t) = exp(min(A-I~, 0))
    O_t = sigmoid(O~) = 0.5*(1 + tanh(O~/2))
    Z_t = tanh(Z~)
    N_t = F_t*Ni + I_t
    C_t = (Ci*F_t + Z_t*I_t)*m + (1-m)*Ci
    H_t = O_t*(C_t/N_t)*m + (1-m)*Hi

Device layout: tokens on partitions, gate columns on the free dim. Activations are
pre-transposed on host to fp16 [dff, tok] so they can be the stationary matmul
operand; weights are the moving operand. The bias is added by a K=1 matmul with a
ones row. All elementwise math is fp32.
"""

import numpy as np

import concourse.bass as bass
import concourse.tile as tile
import concourse.bass_utils as bass_utils
from concourse import bacc, mybir
from concourse.bass import ts, ds

B, P, D, DFF = 256, 64, 512, 1024
NCORES = 8
BL = B // NCORES          # batches per core
TOK = BL * P              # tokens per core (2048)
NT = TOK // 128           # token tiles per core (16)
KH = DFF // 128           # Hi k-tiles (8)
KZ = D // 128             # Zi k-tiles (4)
KT = KH + KZ              # total k-tiles (12)
CH = 2                    # dff column chunks of 512 per gate
CW = 512                  # chunk width

F32 = mybir.dt.float32
F16 = mybir.dt.float16
AF = mybir.ActivationFunctionType
OP = mybir.AluOpType

_CACHE = {}


def _build(repeat: int = 1, recip_approx: bool = True, bias_mm: bool = False,
           gpsimd: bool = True, climit: int = CH, tlimit: int = NT):
    """Build + compile the per-core Bass module. Cached per config.

    bias_mm=False (default) folds the F-gate bias into Mi on the host and
    adds the I/O/Z biases with DVE ops from partition-broadcast tiles;
    bias_mm=True instead seeds PSUM with a K=1 ones-row matmul."""
    key = (repeat, recip_approx, bias_mm, gpsimd, climit, tlimit)
    if key in _CACHE:
        return _CACHE[key]

    nc = bacc.Bacc("TRN2", target_bir_lowering=False, debug=False,
                   num_devices=NCORES)

    hiT = nc.dram_tensor("hiT", [KH, 128, TOK], F16, kind="ExternalInput").ap()
    ziT = nc.dram_tensor("ziT", [KZ, 128, TOK], F16, kind="ExternalInput").ap()
    w = nc.dram_tensor("w", [CH, KT, 128, 4, CW], F16, kind="ExternalInput").ap()
    bias = nc.dram_tensor("bias", [1, CH, 4, CW], F16, kind="ExternalInput").ap()
    biasb = nc.dram_tensor("biasb", [3, CH, CW], F32, kind="ExternalInput").ap()
    mi = nc.dram_tensor("mi", [TOK, DFF], F32, kind="ExternalInput").ap()
    ci = nc.dram_tensor("ci", [TOK, DFF], F32, kind="ExternalInput").ap()
    ni = nc.dram_tensor("ni", [TOK, DFF], F32, kind="ExternalInput").ap()
    hiom = nc.dram_tensor("hiom", [TOK, DFF], F32, kind="ExternalInput").ap()
    mpk = nc.dram_tensor("mpk", [NT, 128, 3], F32, kind="ExternalInput").ap()

    ct = nc.dram_tensor("ct", [TOK, DFF], F32, kind="ExternalOutput").ap()
    mt = nc.dram_tensor("mt", [TOK, DFF], F32, kind="ExternalOutput").ap()
    ht = nc.dram_tensor("ht", [TOK, DFF], F32, kind="ExternalOutput").ap()
    nt = nc.dram_tensor("nt", [TOK, DFF], F32, kind="ExternalOutput").ap()

    with tile.TileContext(nc) as tc:
        with (
            tc.tile_pool(name="singles", bufs=1) as singles,
            tc.tile_pool(name="wpool", bufs=KT + 2) as wpool,
            tc.tile_pool(name="inpool", bufs=2) as inpool,
            tc.tile_pool(name="tmpA", bufs=1) as tmpA,
            tc.tile_pool(name="tmpB", bufs=2) as tmpB,
            tc.tile_pool(name="outp", bufs=2) as outp,
            tc.tile_pool(name="ps", bufs=8, space="PSUM") as pspool,
        ):
            hiT_sb = singles.tile([128, KH, TOK], F16)
            for k in range(KH):
                nc.sync.dma_start(out=hiT_sb[:, k], in_=hiT[k])
            ziT_sb = singles.tile([128, KZ, TOK], F16)
            for k in range(KZ):
                nc.sync.dma_start(out=ziT_sb[:, k], in_=ziT[k])
            mpk_sb = singles.tile([128, NT, 3], F32)
            nc.sync.dma_start(out=mpk_sb, in_=mpk.rearrange("t p c -> p t c"))
            bias_sb = singles.tile([1, CH, 4, CW], F16)
            nc.sync.dma_start(out=bias_sb, in_=bias)
            ones_sb = singles.tile([1, 128], F16)
            nc.vector.memset(ones_sb, 1.0)
            bb_sb = singles.tile([128, 1, CH, CW], F32)
            for cj in range(CH):
                bsl = biasb[0, cj]
                bcast = bass.AP(tensor=bsl.tensor, offset=bsl.offset,
                                ap=[[0, 128]] + list(bsl.ap))
                nc.gpsimd.dma_start(out=bb_sb[:, 0, cj], in_=bcast)

            ett = nc.gpsimd if gpsimd else nc.vector
            for _ in range(repeat):
                for c in range(climit):
                    wk = []
                    for k in range(KT):
                        wt = wpool.tile([128, 4, CW], F16, tag="wk")
                        nc.sync.dma_start(out=wt, in_=w[c, k])
                        wk.append(wt)
                    for t in range(tlimit):
                        rows = ts(t, 128)
                        cols = ds(c * CW, CW)
                        mi_t = inpool.tile([128, CW], F32, tag="mi")
                        nc.sync.dma_start(out=mi_t, in_=mi[rows, cols])
                        ci_t = inpool.tile([128, CW], F32, tag="ci")
                        nc.sync.dma_start(out=ci_t, in_=ci[rows, cols])
                        ni_t = inpool.tile([128, CW], F32, tag="ni")
                        nc.sync.dma_start(out=ni_t, in_=ni[rows, cols])
                        ho_t = inpool.tile([128, CW], F32, tag="ho")
                        nc.sync.dma_start(out=ho_t, in_=hiom[rows, cols])
                        m_ap = mpk_sb[:, t, 0:1]
                        om_ap = mpk_sb[:, t, 1:2]
                        hm_ap = mpk_sb[:, t, 2:3]

                        ps = [pspool.tile([128, CW], F32, tag="ps", name=f"ps{g}")
                              for g in range(4)]
                        # O/Z bias via K=1 ones-row matmul; I-bias added on
                        # GPSIMD after the max; F-bias folded into Mi on host
                        for g in (2, 3):
                            nc.tensor.matmul(ps[g], ones_sb, bias_sb[0:1, c, g],
                                             start=True, stop=False)
                        for k in range(KT):
                            lhsT = (hiT_sb[:, k, rows] if k < KH
                                    else ziT_sb[:, k - KH, rows])
                            for g in range(4):
                                nc.tensor.matmul(ps[g], lhsT, wk[k][:, g],
                                                 start=(k == 0 and g < 2),
                                                 stop=(k == KT - 1))

                        psI, psF, psO, psZ = ps
                        # PSUM readers first so banks free for the next tile
                        A = tmpA.tile([128, CW], F32, tag="A")
                        nc.vector.tensor_add(A, psF, mi_t)
                        Dd = tmpA.tile([128, CW], F32, tag="Dd")
                        nc.vector.tensor_sub(Dd, A, psI)
                        mx = tmpA.tile([128, CW], F32, tag="mx")
                        nc.vector.tensor_max(mx, A, psI)
                        Mt = outp.tile([128, CW], F32, tag="Mt")
                        ett.tensor_add(Mt, mx, bb_sb[:, 0, c])
                        p_ = tmpA.tile([128, CW], F32, tag="p")
                        nc.vector.tensor_scalar_min(p_, Dd, 0.0)
                        pn = tmpA.tile([128, CW], F32, tag="pn")
                        nc.vector.tensor_scalar(pn, Dd, -1.0, 0.0, OP.mult, OP.min)
                        Ft = tmpB.tile([128, CW], F32, tag="Ft")
                        nc.scalar.activation(Ft, p_, AF.Exp)
                        It = tmpB.tile([128, CW], F32, tag="It")
                        nc.scalar.activation(It, pn, AF.Exp)
                        th = tmpB.tile([128, CW], F32, tag="th")
                        nc.scalar.activation(th, psO, AF.Tanh, scale=0.5)
                        Zt = tmpB.tile([128, CW], F32, tag="Zt")
                        nc.scalar.activation(Zt, psZ, AF.Tanh)

                        FN = tmpA.tile([128, CW], F32, tag="FN")
                        ett.tensor_mul(FN, Ft, ni_t)
                        Nt = outp.tile([128, CW], F32, tag="Nt")
                        ett.tensor_add(Nt, FN, It)
                        rec = tmpB.tile([128, CW], F32, tag="rec")
                        if recip_approx:
                            nc.vector.reciprocal_approx_fast(rec, Nt)
                        else:
                            nc.vector.reciprocal(rec, Nt)
                        mF = tmpB.tile([128, CW], F32, tag="mF")
                        nc.vector.tensor_scalar(mF, Ft, m_ap, om_ap, OP.mult, OP.add)
                        p1 = tmpB.tile([128, CW], F32, tag="p1")
                        ett.tensor_mul(p1, ci_t, mF)
                        t2 = tmpB.tile([128, CW], F32, tag="t2")
                        ett.tensor_mul(t2, Zt, It)
                        Ct = outp.tile([128, CW], F32, tag="Ct")
                        nc.vector.scalar_tensor_tensor(Ct, t2, m_ap, p1,
                                                       OP.mult, OP.add)
                        R = tmpB.tile([128, CW], F32, tag="R")
                        ett.tensor_mul(R, Ct, rec)
                        Rh = tmpA.tile([128, CW], F32, tag="Rh")
                        nc.vector.tensor_scalar_mul(Rh, R, hm_ap)
                        u = tmpA.tile([128, CW], F32, tag="u")
                        nc.vector.scalar_tensor_tensor(u, th, 1.0, Rh,
                                                       OP.add, OP.mult)
                        Ht = outp.tile([128, CW], F32, tag="Ht")
                        ett.tensor_add(Ht, u, ho_t)

                        nc.vector.dma_start(out=mt[rows, cols], in_=Mt)
                        nc.vector.dma_start(out=nt[rows, cols], in_=Nt)
                        nc.vector.dma_start(out=ct[rows, cols], in_=Ct)
                        nc.vector.dma_start(out=ht[rows, cols], in_=Ht)

    nc.compile()
    _CACHE[key] = nc
    return nc


def _prep_inputs(inputs):
    """Host-side shard + reformat. Returns per-core input maps."""
    f32, f16 = np.float32, np.float16
    g = {k: np.asarray(v) for k, v in inputs.items()}

    Wh = np.concatenate([g['WI_w'], g['WF_w'], g['WO_w'], g['WZ_w']], axis=1)
    Wz = np.concatenate([g['RI_w'], g['RF_w'], g['RO_w'], g['RZ_w']], axis=1)
    bias = np.concatenate([g['WI_b'] + g['RI_b'], g['WF_b'] + g['RF_b'],
                           g['WO_b'] + g['RO_b'], g['WZ_b'] + g['RZ_b']])
    Wcat = np.vstack([Wh, Wz]).astype(f16)                       # [1536, 4096]
    w_l = np.ascontiguousarray(
        Wcat.reshape(KT, 128, 4, CH, CW).transpose(3, 0, 1, 2, 4))
    bias_l = np.ascontiguousarray(
        bias.astype(f16).reshape(1, 4, CH, CW).transpose(0, 2, 1, 3))
    bI, bF, bO, bZ = bias.reshape(4, DFF).astype(f32)
    biasb_l = np.ascontiguousarray(
        np.stack([bI, bO, bZ]).reshape(3, CH, CW))
    mi_shift = (bF - bI)[None, :]                # folded into Mi on host

    in_maps = []
    for c in range(NCORES):
        sl = slice(c * BL, (c + 1) * BL)
        Hi_c = g['Hi'][sl].reshape(TOK, DFF)
        Zi_c = g['Zi'][sl].reshape(TOK, D)
        m_c = g['m'][sl].reshape(TOK, 1).astype(f32)
        hiT = np.ascontiguousarray(Hi_c.T).astype(f16).reshape(KH, 128, TOK)
        ziT = np.ascontiguousarray(Zi_c.T).astype(f16).reshape(KZ, 128, TOK)
        mpk = np.concatenate([m_c, 1.0 - m_c, 0.5 * m_c],
                             axis=1).astype(f32).reshape(NT, 128, 3)
        in_maps.append({
            "hiT": hiT,
            "ziT": ziT,
            "w": w_l,
            "bias": bias_l,
            "biasb": biasb_l,
            "mi": (g['Mi'][sl].reshape(TOK, DFF) + mi_shift).astype(f32),
            "ci": np.ascontiguousarray(g['Ci'][sl].reshape(TOK, DFF), f32),
            "ni": np.ascontiguousarray(g['Ni'][sl].reshape(TOK, DFF), f32),
            "hiom": ((1.0 - m_c) * Hi_c).astype(f32),
            "mpk": mpk,
        })
    return in_maps


def _gather(results):
    def cat(name):
        full = np.concatenate(
            [results[c][name].reshape(BL, P, DFF) for c in range(NCORES)],
            axis=0)
        return np.ascontiguousarray(full, dtype=np.float32)
    return cat("ct"), cat("mt"), cat("ht"), cat("nt")


def kernel(**inputs):
    nc = _build(repeat=1)
    in_maps = _prep_inputs(inputs)
    res = bass_utils.run_bass_kernel_spmd(nc, in_maps,
                                          core_ids=list(range(NCORES)))
    return _gather(res.results)


# revision 19
# speedup vs baseline: 9.8947x; 9.8947x over previous
"""Custom LSTM-cell kernel for Trainium2, data-parallel over batch on 8 NeuronCores.

Math (per token, elementwise over dff except the two GEMMs):
    gates = Hi @ Wh + Zi @ Wz + bias         # [tok, 4*dff], gate order I|F|O|Z
    A   = F~ + Mi
    M_t = max(A, I~)
    I_t = exp(I~ - M_t) = exp(min(-(A-I~), 0))
    F_t = exp(A  - M_t) = exp(min(A-I~, 0))
    O_t = sigmoid(O~) = 0.5*(1 + tanh(O~/2))
    Z_t = tanh(Z~)
    N_t = F_t*Ni + I_t
    C_t = (Ci*F_t + Z_t*I_t)*m + (1-m)*Ci
    H_t = O_t*(C_t/N_t)*m + (1-m)*Hi

Device layout: tokens on partitions, gate columns on the free dim. Activations are
pre-transposed on host to fp16 [dff, tok] so they can be the stationary matmul
operand; weights are the moving operand (fp16 inputs, fp32 PSUM accumulate).
Biases: F-gate bias is folded into Mi on the host (Mi + bF - bI), the I-gate bias
is added on GPSIMD after the max (M_t = max(..) + bI), and the O/Z biases are
seeded into PSUM by a K=1 ones-row matmul. All elementwise math is fp32; the
division uses the 18-bit reciprocal_approx_fast. Work is spread so the 1600
matmuls on the PE stay the critical path: PSUM-reading ops and the M_t/exp-arg
chain on DVE, transcendentals on ScalarE (one table set: exp+tanh, sigmoid via
tanh), and 6 of the remaining elementwise ops per tile on GPSIMD.
"""

import numpy as np

import concourse.bass as bass
import concourse.tile as tile
import concourse.bass_utils as bass_utils
from concourse import bacc, mybir
from concourse.bass import ts, ds

B, P, D, DFF = 256, 64, 512, 1024
NCORES = 8
BL = B // NCORES          # batches per core
TOK = BL * P              # tokens per core (2048)
NT = TOK // 128           # token tiles per core (16)
KH = DFF // 128           # Hi k-tiles (8)
KZ = D // 128             # Zi k-tiles (4)
KT = KH + KZ              # total k-tiles (12)
CH = 2                    # dff column chunks of 512 per gate
CW = 512                  # chunk width

F32 = mybir.dt.float32
F16 = mybir.dt.float16
AF = mybir.ActivationFunctionType
OP = mybir.AluOpType

_CACHE = {}


def _build(repeat: int = 1, recip_approx: bool = True, bias_mm: bool = False,
           gpsimd: bool = True, climit: int = CH, tlimit: int = NT):
    """Build + compile the per-core Bass module. Cached per config.

    bias_mm=False (default) folds the F-gate bias into Mi on the host and
    adds the I/O/Z biases with DVE ops from partition-broadcast tiles;
    bias_mm=True instead seeds PSUM with a K=1 ones-row matmul."""
    key = (repeat, recip_approx, bias_mm, gpsimd, climit, tlimit)
    if key in _CACHE:
        return _CACHE[key]

    nc = bacc.Bacc("TRN2", target_bir_lowering=False, debug=False,
                   num_devices=NCORES)

    hiT = nc.dram_tensor("hiT", [KH, 128, TOK], F16, kind="ExternalInput").ap()
    ziT = nc.dram_tensor("ziT", [KZ, 128, TOK], F16, kind="ExternalInput").ap()
    w = nc.dram_tensor("w", [CH, KT, 128, 4, CW], F16, kind="ExternalInput").ap()
    bias = nc.dram_tensor("bias", [1, CH, 4, CW], F16, kind="ExternalInput").ap()
    biasb = nc.dram_tensor("biasb", [3, CH, CW], F32, kind="ExternalInput").ap()
    mi = nc.dram_tensor("mi", [TOK, DFF], F32, kind="ExternalInput").ap()
    ci = nc.dram_tensor("ci", [TOK, DFF], F32, kind="ExternalInput").ap()
    ni = nc.dram_tensor("ni", [TOK, DFF], F32, kind="ExternalInput").ap()
    hiom = nc.dram_tensor("hiom", [TOK, DFF], F32, kind="ExternalInput").ap()
    mpk = nc.dram_tensor("mpk", [NT, 128, 3], F32, kind="ExternalInput").ap()

    ct = nc.dram_tensor("ct", [TOK, DFF], F32, kind="ExternalOutput").ap()
    mt = nc.dram_tensor("mt", [TOK, DFF], F32, kind="ExternalOutput").ap()
    ht = nc.dram_tensor("ht", [TOK, DFF], F32, kind="ExternalOutput").ap()
    nt = nc.dram_tensor("nt", [TOK, DFF], F32, kind="ExternalOutput").ap()

    with tile.TileContext(nc) as tc:
        with (
            tc.tile_pool(name="singles", bufs=1) as singles,
            tc.tile_pool(name="wpool", bufs=KT + 5) as wpool,
            tc.tile_pool(name="inpool", bufs=2) as inpool,
            tc.tile_pool(name="tmpA", bufs=1) as tmpA,
            tc.tile_pool(name="tmpB", bufs=2) as tmpB,
            tc.tile_pool(name="outp", bufs=2) as outp,
            tc.tile_pool(name="ps", bufs=8, space="PSUM") as pspool,
        ):
            hiT_sb = singles.tile([128, KH, TOK], F16)
            for k in range(KH):
                nc.sync.dma_start(out=hiT_sb[:, k], in_=hiT[k])
            ziT_sb = singles.tile([128, KZ, TOK], F16)
            for k in range(KZ):
                nc.sync.dma_start(out=ziT_sb[:, k], in_=ziT[k])
            mpk_sb = singles.tile([128, NT, 3], F32)
            nc.sync.dma_start(out=mpk_sb, in_=mpk.rearrange("t p c -> p t c"))
            bias_sb = singles.tile([1, CH, 4, CW], F16)
            nc.sync.dma_start(out=bias_sb, in_=bias)
            ones_sb = singles.tile([1, 128], F16)
            nc.vector.memset(ones_sb, 1.0)
            bb_sb = singles.tile([128, 1, CH, CW], F32)
            for cj in range(CH):
                bsl = biasb[0, cj]
                bcast = bass.AP(tensor=bsl.tensor, offset=bsl.offset,
                                ap=[[0, 128]] + list(bsl.ap))
                nc.gpsimd.dma_start(out=bb_sb[:, 0, cj], in_=bcast)

            ett = nc.gpsimd if gpsimd else nc.vector
            for _ in range(repeat):
                for c in range(climit):
                    wk = []
                    for k in range(KT):
                        wt = wpool.tile([128, 4, CW], F16, tag="wk")
                        nc.sync.dma_start(out=wt, in_=w[c, k])
                        wk.append(wt)
                    for t in range(tlimit):
                        rows = ts(t, 128)
                        cols = ds(c * CW, CW)
                        mi_t = inpool.tile([128, CW], F32, tag="mi")
                        nc.sync.dma_start(out=mi_t, in_=mi[rows, cols])
                        ci_t = inpool.tile([128, CW], F32, tag="ci")
                        nc.sync.dma_start(out=ci_t, in_=ci[rows, cols])
                        ni_t = inpool.tile([128, CW], F32, tag="ni")
                        nc.sync.dma_start(out=ni_t, in_=ni[rows, cols])
                        ho_t = inpool.tile([128, CW], F32, tag="ho")
                        nc.sync.dma_start(out=ho_t, in_=hiom[rows, cols])
                        m_ap = mpk_sb[:, t, 0:1]
                        om_ap = mpk_sb[:, t, 1:2]
                        hm_ap = mpk_sb[:, t, 2:3]

                        ps = [pspool.tile([128, CW], F32, tag="ps", name=f"ps{g}")
                              for g in range(4)]
                        # O/Z bias via K=1 ones-row matmul; I-bias added on
                        # GPSIMD after the max; F-bias folded into Mi on host
                        for g in (2, 3):
                            nc.tensor.matmul(ps[g], ones_sb, bias_sb[0:1, c, g],
                                             start=True, stop=False)
                        for k in range(KT):
                            lhsT = (hiT_sb[:, k, rows] if k < KH
                                    else ziT_sb[:, k - KH, rows])
                            for g in range(4):
                                nc.tensor.matmul(ps[g], lhsT, wk[k][:, g],
                                                 start=(k == 0 and g < 2),
                                                 stop=(k == KT - 1))

                        psI, psF, psO, psZ = ps
                        # PSUM readers first so banks free for the next tile
                        A = tmpA.tile([128, CW], F32, tag="A")
                        nc.vector.tensor_add(A, psF, mi_t)
                        Dd = tmpA.tile([128, CW], F32, tag="Dd")
                        nc.vector.tensor_sub(Dd, A, psI)
                        mx = tmpA.tile([128, CW], F32, tag="mx")
                        nc.vector.tensor_max(mx, A, psI)
                        Mt = outp.tile([128, CW], F32, tag="Mt")
                        ett.tensor_add(Mt, mx, bb_sb[:, 0, c])
                        p_ = tmpA.tile([128, CW], F32, tag="p")
                        nc.vector.tensor_scalar_min(p_, Dd, 0.0)
                        pn = tmpA.tile([128, CW], F32, tag="pn")
                        nc.vector.tensor_scalar(pn, Dd, -1.0, 0.0, OP.mult, OP.min)
                        Ft = tmpB.tile([128, CW], F32, tag="Ft")
                        nc.scalar.activation(Ft, p_, AF.Exp)
                        It = tmpB.tile([128, CW], F32, tag="It")
                        nc.scalar.activation(It, pn, AF.Exp)
                        th = tmpB.tile([128, CW], F32, tag="th")
                        nc.scalar.activation(th, psO, AF.Tanh, scale=0.5)
                        Zt = tmpB.tile([128, CW], F32, tag="Zt")
                        nc.scalar.activation(Zt, psZ, AF.Tanh)

                        FN = tmpA.tile([128, CW], F32, tag="FN")
                        ett.tensor_mul(FN, Ft, ni_t)
                        Nt = outp.tile([128, CW], F32, tag="Nt")
                        ett.tensor_add(Nt, FN, It)
                        rec = tmpB.tile([128, CW], F32, tag="rec")
                        if recip_approx:
                            nc.vector.reciprocal_approx_fast(rec, Nt)
                        else:
                            nc.vector.reciprocal(rec, Nt)
                        mF = tmpA.tile([128, CW], F32, tag="mF")
                        nc.vector.tensor_scalar(mF, Ft, m_ap, om_ap, OP.mult, OP.add)
                        p1 = tmpA.tile([128, CW], F32, tag="p1")
                        ett.tensor_mul(p1, ci_t, mF)
                        t2 = tmpA.tile([128, CW], F32, tag="t2")
                        ett.tensor_mul(t2, Zt, It)
                        Ct = outp.tile([128, CW], F32, tag="Ct")
                        nc.vector.scalar_tensor_tensor(Ct, t2, m_ap, p1,
                                                       OP.mult, OP.add)
                        R = tmpA.tile([128, CW], F32, tag="R")
                        ett.tensor_mul(R, Ct, rec)
                        Rh = tmpA.tile([128, CW], F32, tag="Rh")
                        nc.vector.tensor_scalar_mul(Rh, R, hm_ap)
                        u = tmpA.tile([128, CW], F32, tag="u")
                        nc.vector.scalar_tensor_tensor(u, th, 1.0, Rh,
                                                       OP.add, OP.mult)
                        Ht = outp.tile([128, CW], F32, tag="Ht")
                        ett.tensor_add(Ht, u, ho_t)

                        nc.sync.dma_start(out=mt[rows, cols], in_=Mt)
                        nc.sync.dma_start(out=nt[rows, cols], in_=Nt)
                        nc.sync.dma_start(out=ct[rows, cols], in_=Ct)
                        nc.sync.dma_start(out=ht[rows, cols], in_=Ht)

    nc.compile()
    _CACHE[key] = nc
    return nc


def _prep_inputs(inputs):
    """Host-side shard + reformat. Returns per-core input maps."""
    f32, f16 = np.float32, np.float16
    g = {k: np.asarray(v) for k, v in inputs.items()}

    Wh = np.concatenate([g['WI_w'], g['WF_w'], g['WO_w'], g['WZ_w']], axis=1)
    Wz = np.concatenate([g['RI_w'], g['RF_w'], g['RO_w'], g['RZ_w']], axis=1)
    bias = np.concatenate([g['WI_b'] + g['RI_b'], g['WF_b'] + g['RF_b'],
                           g['WO_b'] + g['RO_b'], g['WZ_b'] + g['RZ_b']])
    Wcat = np.vstack([Wh, Wz]).astype(f16)                       # [1536, 4096]
    w_l = np.ascontiguousarray(
        Wcat.reshape(KT, 128, 4, CH, CW).transpose(3, 0, 1, 2, 4))
    bias_l = np.ascontiguousarray(
        bias.astype(f16).reshape(1, 4, CH, CW).transpose(0, 2, 1, 3))
    bI, bF, bO, bZ = bias.reshape(4, DFF).astype(f32)
    biasb_l = np.ascontiguousarray(
        np.stack([bI, bO, bZ]).reshape(3, CH, CW))
    mi_shift = (bF - bI)[None, :]                # folded into Mi on host

    in_maps = []
    for c in range(NCORES):
        sl = slice(c * BL, (c + 1) * BL)
        Hi_c = g['Hi'][sl].reshape(TOK, DFF)
        Zi_c = g['Zi'][sl].reshape(TOK, D)
        m_c = g['m'][sl].reshape(TOK, 1).astype(f32)
        hiT = np.ascontiguousarray(Hi_c.T).astype(f16).reshape(KH, 128, TOK)
        ziT = np.ascontiguousarray(Zi_c.T).astype(f16).reshape(KZ, 128, TOK)
        mpk = np.concatenate([m_c, 1.0 - m_c, 0.5 * m_c],
                             axis=1).astype(f32).reshape(NT, 128, 3)
        in_maps.append({
            "hiT": hiT,
            "ziT": ziT,
            "w": w_l,
            "bias": bias_l,
            "biasb": biasb_l,
            "mi": (g['Mi'][sl].reshape(TOK, DFF) + mi_shift).astype(f32),
            "ci": np.ascontiguousarray(g['Ci'][sl].reshape(TOK, DFF), f32),
            "ni": np.ascontiguousarray(g['Ni'][sl].reshape(TOK, DFF), f32),
            "hiom": ((1.0 - m_c) * Hi_c).astype(f32),
            "mpk": mpk,
        })
    return in_maps


def _gather(results):
    def cat(name):
        full = np.concatenate(
            [results[c][name].reshape(BL, P, DFF) for c in range(NCORES)],
            axis=0)
        return np.ascontiguousarray(full, dtype=np.float32)
    return cat("ct"), cat("mt"), cat("ht"), cat("nt")


def kernel(**inputs):
    nc = _build(repeat=1)
    in_maps = _prep_inputs(inputs)
    res = bass_utils.run_bass_kernel_spmd(nc, in_maps,
                                          core_ids=list(range(NCORES)))
    return _gather(res.results)

